# revision 32
# baseline (speedup 1.0000x reference)
"""Tensor-parallel multi-head attention for 8 Trainium2 NeuronCores.

Sharding (TP8 over heads): core c owns heads {2c, 2c+1} (128 q/k/v features)
and computes them for BOTH batch elements.  The out-projection is
TOKEN-sharded: after attention for batch b finishes, one 8-core AllToAll
(512 KB/core wire) redistributes context so core c holds ALL 1024 context
features for its 256-token slice of batch b; each core then computes the
full out-proj for its tokens with the complete (unsharded) Wo.  This
replaces the previous six serialized AllGathers (8 MB/core gathered,
~122 us of collective time) with two ~10 us AllToAlls, the first of which
fully overlaps batch 1's attention.

Per-core dataflow (activations kept transposed, [feature, token]):
  qT/kT/vT = W.T-chunks @ xT          (PE, bf16, fp32 PSUM accum)
  v        = PE-transpose(vT)          (with an appended ones-column)
  sT[k,q]  = kT-block.T @ qT           (causal: upper-right blocks skipped)
  aT       = exp(sT/8 + mask_bias)     (ACT from PSUM; safe without
                                        max-subtraction: scores ~ N(0,1))
  ctxT;sum = [v|1].T @ aT              (ones row gives the softmax denom)
  ctxT    *= 1/sum                     (per-q-block, as soon as its k-loop
                                        completes; batch-0 denominators
                                        broadcast on GpSimd -- its queue slot
                                        precedes every collective trigger --
                                        batch 1 via a bf16 PE matmul so it
                                        can never jam behind a collective's
                                        completion wait)
  AllToAll ctxT per batch; out-proj for my 256-token slice per batch

Engine queues are FIFO, so emission order is execution order per engine.
The attention stream is software-pipelined: the ctx matmuls of k-chunk step
N are emitted several steps behind its score matmuls, giving the Scalar
engine time to produce exp(scores) before the Tensor engine's queue reaches
the ctx matmul that consumes them (deep lag keeps the PE stream dense so
the HAM clock gate stays released).  Two (head, batch) pairs are
interleaved 2:1 so PSUM context accumulators fit alongside the
double-buffered score tiles.  Batch 0's out-proj is emitted after the
batch-1 AllToAll trigger so it fills the PE while that collective is in
flight; batch 1's out-proj runs as soon as its gather lands, streaming its
bf16 writebacks per out-chunk.  Collective staging DMAs are bulk
(not spread through the stream) and output writebacks are ordered behind
the gather loads, so nothing contends the SDMA engines while a collective
is on the wire; nothing except collective triggers rides the GpSimd queue,
whose completion-waits would otherwise jam work queued behind them.
Host side only reshapes/concatenates shards (dtype prep of inputs aside);
the gathered feature order is already the global head order, so no host
permutation is needed.
"""

import sys
from collections import deque

for _p in ("/opt/trn_rl_repo",):
    if _p not in sys.path:
        sys.path.append(_p)

import numpy as np
import ml_dtypes

import concourse.bass as bass  # noqa: F401
import concourse.mybir as mybir
import concourse.tile as tile
from concourse import bacc, bass_utils
from concourse.masks import make_identity, make_upper_triangular

BF16 = mybir.dt.bfloat16
F32 = mybir.dt.float32
Exp = mybir.ActivationFunctionType.Exp

B, S, D = 2, 2048, 1024
T = B * S            # 4096 tokens across batches
H, DH = 16, 64
NCORES = 8
HPC = H // NCORES    # heads per core = 2
F = HPC * DH         # features per core = 128
KC = S // 128        # 16 k-chunks per batch
TOK = S // NCORES    # tokens per core per batch for the out-proj = 256

PAIRS = [(0, 0), (1, 0), (0, 1), (1, 1)]

_CACHED = {}


def _build(with_bias: bool):
    nc = bacc.Bacc(
        "TRN2",
        target_bir_lowering=False,
        debug=False,
        enable_asserts=True,
        num_devices=NCORES,
    )
    xT_d = nc.dram_tensor("xT", [D, T], BF16, kind="ExternalInput").ap()
    wqT_d = nc.dram_tensor("wqT", [D, F], BF16, kind="ExternalInput").ap()
    wkT_d = nc.dram_tensor("wkT", [D, F], BF16, kind="ExternalInput").ap()
    wvT_d = nc.dram_tensor("wvT", [D, F], BF16, kind="ExternalInput").ap()
    woT_d = nc.dram_tensor("woT", [D, D], BF16, kind="ExternalInput").ap()
    b_d = {}
    if with_bias:
        for nm, width in (("bq", F), ("bk", F), ("bv", F), ("bo", D)):
            b_d[nm] = nc.dram_tensor(nm, [1, width], BF16, kind="ExternalInput").ap()
    maskb_d = nc.dram_tensor("maskb", [128, B * KC], F32, kind="ExternalInput").ap()
    outT_d = nc.dram_tensor("outT", [D, B * TOK], BF16, kind="ExternalOutput").ap()

    with tile.TileContext(nc) as tc:
        with (
            tc.tile_pool(name="singles", bufs=1) as sg,
            tc.tile_pool(name="att", bufs=8) as att_pool,
            tc.tile_pool(name="psA", bufs=2, space="PSUM") as psA,
            tc.tile_pool(name="psB", bufs=4, space="PSUM") as psB,
            tc.tile_pool(name="dram", bufs=1, space="DRAM") as dram,
        ):
            # ---- constants -------------------------------------------------
            ident = sg.tile([128, 128], BF16, name="ident")
            make_identity(nc, ident)
            trimask = sg.tile([128, 128], BF16, name="trimask")
            make_upper_triangular(nc, trimask, val=1.0, diag=True)
            ones64b = sg.tile([1, 64], BF16, name="ones64b")
            nc.vector.memset(ones64b, 1.0)
            if with_bias:
                ones512 = sg.tile([1, 512], BF16, name="ones512")
                nc.vector.memset(ones512, 1.0)

            # ---- load inputs (split for early start) -----------------------
            w_sb = {}
            for nm, dd, fo in (("v", wvT_d, F), ("k", wkT_d, F), ("q", wqT_d, F)):
                w_sb[nm] = sg.tile([128, 8, fo], BF16, name=f"w{nm}T_sb")
                nc.sync.dma_start(w_sb[nm], dd.rearrange("(o p) f -> p o f", p=128))
            b_sb = {}
            if with_bias:
                for nm, width in (("bq", F), ("bk", F), ("bv", F), ("bo", D)):
                    b_sb[nm] = sg.tile([1, width], BF16, name=f"{nm}_sb")
                    nc.sync.dma_start(b_sb[nm], b_d[nm])

            # persistent activations first, xT last (freed first: LIFO stack)
            qT_sb, qT_free = tc.tile([128, T], BF16, name="qT_sb")
            kT_sb, kT_free = tc.tile([128, T], BF16, name="kT_sb")
            ctxT_sb, ctxT_free = tc.tile([64, HPC, T], BF16, name="ctxT_sb")
            vT_sb, vT_free = tc.tile([128, T], BF16, name="vT_sb")
            xT_sb, xT_free = tc.tile([128, 8, T], BF16, name="xT_sb")
            xT_r = xT_d.rearrange("(o p) f -> p o f", p=128)
            # ki-major for the first half-pair (projection consumes ki-order),
            # second half-pair afterwards (its projections run later)
            for hp in range(2):
                for ki in range(8):
                    for half in (2 * hp, 2 * hp + 1):
                        cs = half * 1024
                        nc.sync.dma_start(
                            xT_sb[:, ki, cs:cs + 1024], xT_r[:, ki, cs:cs + 1024]
                        )
            # maskb (attention-phase) and Wo (mid-kernel) land after x
            maskb_sb = sg.tile([128, B * KC], F32, name="maskb_sb")
            nc.sync.dma_start(maskb_sb, maskb_d)
            w_sb["o"] = sg.tile([128, 8, D], BF16, name="woT_sb")
            nc.sync.dma_start(w_sb["o"], woT_d.rearrange("(o p) f -> p o f", p=128))

            # ---- HAM warmup: dependency-free matmuls fill the otherwise
            # idle PE while the first x/weight DMAs stream in, so the clock
            # gate is already released when the projections start ----------
            warm = psA.tile([128, 512], F32, tag="work", name="warm")
            for _ in range(36):
                nc.tensor.matmul(
                    warm[:, 0:128], lhsT=ident, rhs=ident, start=True, stop=True
                )

            # ---- projections (ki-outer: 4 matmuls per weight load) ---------
            def project(w, bias, dst, which, hp):
                pss = [
                    psA.tile(
                        [128, 1024], F32, tag="work",
                        name=f"p_{which}_{2 * hp + i}",
                    )
                    for i in range(2)
                ]
                for ki in range(8):
                    for i in range(2):
                        half = 2 * hp + i
                        for nb in range(2):
                            cs = half * 1024 + nb * 512
                            nc.tensor.matmul(
                                pss[i][:, nb * 512:nb * 512 + 512],
                                lhsT=w[:, ki, :],
                                rhs=xT_sb[:, ki, cs:cs + 512],
                                start=(ki == 0),
                                stop=(ki == 7 and not with_bias),
                            )
                for i in range(2):
                    half = 2 * hp + i
                    if with_bias:
                        for nb in range(2):
                            nc.tensor.matmul(
                                pss[i][:, nb * 512:nb * 512 + 512],
                                lhsT=bias[0:1, :],
                                rhs=ones512[0:1, :],
                                start=False,
                                stop=True,
                            )
                    nc.vector.tensor_copy(
                        dst[:, half * 1024:half * 1024 + 1024], pss[i]
                    )

            def project_vk(hp):
                vt = [
                    psA.tile(
                        [128, 1024], F32, tag="work", name=f"p_v_{2 * hp + i}"
                    )
                    for i in range(2)
                ]
                kt = [
                    psB.tile(
                        [128, 512], F32, tag="ctx", name=f"p_k_{2 * hp}_{j}"
                    )
                    for j in range(4)
                ]
                for ki in range(8):
                    for i in range(2):
                        half = 2 * hp + i
                        for nb in range(2):
                            cs = half * 1024 + nb * 512
                            nc.tensor.matmul(
                                vt[i][:, nb * 512:nb * 512 + 512],
                                lhsT=w_sb["v"][:, ki, :],
                                rhs=xT_sb[:, ki, cs:cs + 512],
                                start=(ki == 0),
                                stop=(ki == 7 and not with_bias),
                            )
                            nc.tensor.matmul(
                                kt[2 * i + nb],
                                lhsT=w_sb["k"][:, ki, :],
                                rhs=xT_sb[:, ki, cs:cs + 512],
                                start=(ki == 0),
                                stop=(ki == 7 and not with_bias),
                            )
                for i in range(2):
                    half = 2 * hp + i
                    for nb in range(2):
                        cs = half * 1024 + nb * 512
                        if with_bias:
                            nc.tensor.matmul(
                                vt[i][:, nb * 512:nb * 512 + 512],
                                lhsT=b_sb["bv"][0:1, :],
                                rhs=ones512[0:1, :],
                                start=False,
                                stop=True,
                            )
                            nc.tensor.matmul(
                                kt[2 * i + nb],
                                lhsT=b_sb["bk"][0:1, :],
                                rhs=ones512[0:1, :],
                                start=False,
                                stop=True,
                            )
                        nc.vector.tensor_copy(
                            kT_sb[:, cs:cs + 512], kt[2 * i + nb]
                        )
                    nc.vector.tensor_copy(
                        vT_sb[:, half * 1024:half * 1024 + 1024], vt[i]
                    )

            # v+k interleaved (DMA-paced), then q (x already resident)
            for hp in range(2):
                project_vk(hp)
                project(w_sb["q"], b_sb.get("bq"), qT_sb, "q", hp)

            # ---- transpose v into [token, feat] blocks with ones column ----
            v_ones = sg.tile([128, B * KC, HPC, DH + 1], BF16, name="v_ones")
            nc.vector.memset(v_ones, 1.0)
            for tb in range(B * KC):
                pt = psB.tile([128, 128], BF16, tag="ctx", name=f"vt_{tb}")
                nc.tensor.transpose(pt, vT_sb[:, tb * 128:tb * 128 + 128], ident)
                for h in range(HPC):
                    nc.vector.tensor_copy(
                        v_ones[:, tb, h, 0:DH], pt[:, h * 64:h * 64 + 64]
                    )

            xT_free()
            vT_free()

            # ---- attention -------------------------------------------------
            sums_b = sg.tile([1, 2 * S], BF16, name="sums_b")
            sums_f = sg.tile([1, 2 * S], F32, name="sums_f")
            rec1_sb = sg.tile([1, 2 * S], F32, name="rec1_sb")
            rec_sb = sg.tile([64, 2048], F32, name="rec_sb")
            # out-proj token columns: [0:256) = my batch-0 token slice,
            # [256:512) = my batch-1 token slice
            outT_sb, outT_free = tc.tile([128, 8, B * TOK], BF16, name="outT_sb")
            ctxA_sb, ctxA_free = tc.tile([128, 8, B * TOK], BF16, name="ctxA_sb")

            # ships: key = batch index, one AllToAll per batch (256
            # tok/core); the batch-0 ship fully overlaps batch-1 attention
            SHIPS = {0: (0, 0, S), 1: (1, 0, S)}
            a2a_in = {}
            a2a_out = {}
            for key, (_b, _q0, qn) in SHIPS.items():
                tk = qn // NCORES
                a2a_in[key] = dram.tile([NCORES * F, tk], BF16, name=f"a2ai_{key}")
                a2a_out[key] = dram.tile([NCORES * F, tk], BF16, name=f"a2ao_{key}")

            # normalize is split in two: the sums copy is emitted with the
            # final ctx matmul; the broadcast matmul + reciprocal + scale run
            # one pipeline round later so the PE never waits on the Vector
            # queue.  The scaled ctxT q-block is shipped to the AllToAll
            # staging buffer immediately, so the collective trigger only
            # waits on the final 64 KB piece.
            def normalize_a(h, b, lane, qb, ctx_tile):
                so = lane * S + qb * 512
                if b == 0:
                    nc.vector.tensor_copy(
                        sums_f[0:1, so:so + 512], ctx_tile[DH:DH + 1, :]
                    )
                    nc.vector.reciprocal_approx_fast(
                        rec1_sb[0:1, so:so + 512], sums_f[0:1, so:so + 512]
                    )
                else:
                    nc.vector.tensor_copy(
                        sums_b[0:1, so:so + 512], ctx_tile[DH:DH + 1, :]
                    )

            def normalize_b(h, b, lane, qb, ctx_tile):
                t0 = b * S
                so = lane * S + qb * 512
                ro = lane * 1024 + (qb % 2) * 512
                # batch 0: GpSimd partition_broadcast (its queue slot is
                # ahead of every collective trigger, so it can never jam).
                # batch 1: broadcast via a tiny bf16 matmul -- its GpSimd
                # slot would sit behind the batch-0 AllToAll trigger's
                # completion wait, jamming the normalize chain whenever that
                # collective runs long (measured 17 us PE stalls).
                if b == 0:
                    nc.gpsimd.partition_broadcast(
                        rec_sb[:, ro:ro + 512], rec1_sb[0:1, so:so + 512]
                    )
                else:
                    bc = psA.tile(
                        [128, 512], F32, tag="work", name=f"bc_{h}_{b}_{qb}"
                    )
                    nc.tensor.matmul(
                        bc[0:64, :],
                        lhsT=ones64b[0:1, :],
                        rhs=sums_b[0:1, so:so + 512],
                        start=True,
                        stop=True,
                    )
                    nc.vector.reciprocal_approx_fast(
                        rec_sb[:, ro:ro + 512], bc[0:64, :]
                    )
                nc.vector.tensor_mul(
                    ctxT_sb[:, h, t0 + qb * 512:t0 + qb * 512 + 512],
                    ctx_tile[0:DH, :],
                    rec_sb[:, ro:ro + 512],
                )

            def scores_part(h, b, kc, qlo, qhi):
                """Emit score matmuls + exp for one k-chunk; returns the
                attention-weights tile for the ctx part."""
                po = 64 * h
                t0 = b * S
                q0 = kc * 128
                lo = max(q0, qlo)
                w = qhi - lo
                kT_blk = kT_sb[po:po + 64, t0 + q0:t0 + q0 + 128]
                st = psA.tile(
                    [128, 1024], F32, tag="work", name=f"st_{h}_{b}_{kc}_{qlo}"
                )
                c = lo
                while c < qhi:
                    c2 = min(qhi, (c // 512 + 1) * 512)
                    nc.tensor.matmul(
                        st[:, c - qlo:c2 - qlo],
                        lhsT=kT_blk,
                        rhs=qT_sb[po:po + 64, t0 + c:t0 + c2],
                        start=True,
                        stop=True,
                    )
                    c = c2
                at = att_pool.tile([128, 1024], BF16, tag="att")
                nc.scalar.activation(
                    at[:, 0:w],
                    st[:, lo - qlo:qhi - qlo],
                    Exp,
                    bias=maskb_sb[:, b * KC + kc:b * KC + kc + 1],
                    scale=0.125,
                )
                if lo == q0:  # diagonal 128-block: causal interior
                    nc.vector.tensor_mul(at[:, 0:128], at[:, 0:128], trimask)
                return at

            def ctx_part(h, b, lane, kc, qlo, qhi, ctx_ps, at):
                t0 = b * S
                q0 = kc * 128
                lo = max(q0, qlo)
                c = lo
                while c < qhi:
                    qb = c // 512
                    c2 = min(qhi, (qb + 1) * 512)
                    nc.tensor.matmul(
                        ctx_ps[qb][0:DH + 1, c - qb * 512:c2 - qb * 512],
                        lhsT=v_ones[:, b * KC + kc, h, :],
                        rhs=at[:, c - lo:c2 - lo],
                        start=(kc == 0),
                        stop=(kc == 4 * qb + 3),
                    )
                    c = c2
                if kc >= 3 and (kc - 3) % 4 == 0:
                    qb_done = (kc - 3) // 4
                    if qlo <= qb_done * 512 < qhi:
                        normalize_a(h, b, lane, qb_done, ctx_ps[qb_done])
                        pending.append(
                            lambda h=h, b=b, lane=lane, qb=qb_done,
                            ct=ctx_ps[qb_done]: normalize_b(h, b, lane, qb, ct)
                        )

            # out-proj column ranges per ship key
            OCOL = {0: 0, 1: 256}

            def ship_a2a(key):
                # scatter my ctxT for this token range into 8 shards (one per
                # peer): shard j = my 128 features for its j-th token slice.
                # Bulk DMA here (not staged per q-block): staging DMAs spread
                # through the attention stream land inside an earlier ship's
                # AllToAll window and contend for the SDMA engines, measured
                # to cut the collective's wire rate ~4x.
                b, q0, qn = SHIPS[key]
                tk = qn // NCORES
                dst = a2a_in[key].rearrange(
                    "(j h p) t -> p h j t", j=NCORES, h=HPC, p=DH
                )
                src = ctxT_sb[:, :, b * S + q0:b * S + q0 + qn].rearrange(
                    "p h (j t) -> p h j t", j=NCORES
                )
                nc.sync.dma_start(dst[:, 0], src[:, 0])
                nc.scalar.dma_start(dst[:, 1], src[:, 1])
                nc.gpsimd.collective_compute(
                    "AllToAll",
                    mybir.AluOpType.bypass,
                    replica_groups=[list(range(NCORES))],
                    ins=[a2a_in[key].opt()],
                    outs=[a2a_out[key].opt()],
                )
                # my slice: all 1024 features (global head order) x tk
                # tokens; the final ship's load is split so the out-proj can
                # start accumulating after the first half lands
                r = a2a_out[key].rearrange("(k p) t -> p k t", p=128)
                co = OCOL[key]
                nc.sync.dma_start(ctxA_sb[:, :, co:co + tk], r)

            outv = outT_d.rearrange("(o p) t -> p o t", p=128)

            def outproj_chunk(key, half, stream_wb=False):
                _b, _q0, qn = SHIPS[key]
                tk = qn // NCORES
                co = OCOL[key]
                ps = psA.tile(
                    [128, 4, tk], F32, tag="work", name=f"op_{key}_{half}"
                )
                tslc = slice(co, co + tk)
                for i in range(4):
                    oc = half * 4 + i
                    for ki in range(8):
                        nc.tensor.matmul(
                            ps[:, i, :],
                            lhsT=w_sb["o"][:, ki, oc * 128:oc * 128 + 128],
                            rhs=ctxA_sb[:, ki, tslc],
                            start=(ki == 0),
                            stop=(ki == 7 and not with_bias),
                        )
                    if with_bias:
                        nc.tensor.matmul(
                            ps[:, i, :],
                            lhsT=b_sb["bo"][0:1, oc * 128:oc * 128 + 128],
                            rhs=ones512[0:1, 0:tk],
                            start=False,
                            stop=True,
                        )
                    nc.vector.tensor_copy(outT_sb[:, oc, tslc], ps[:, i, :])
                    if stream_wb:
                        nc.sync.dma_start(
                            outv[:, oc, tslc], outT_sb[:, oc, tslc]
                        )

            def writeback(lo, hi):
                # sync queue: ordered behind the ctxA gather loads, so the
                # writes never overlap an in-flight collective
                nc.sync.dma_start(outv[:, :, lo:hi], outT_sb[:, :, lo:hi])

            # ---- build the interleaved, software-pipelined stream ---------
            lanes = {p: i % 2 for i, p in enumerate(PAIRS)}
            ctx_tiles = {}

            def get_ctx(p, qb):
                if (p, qb) not in ctx_tiles:
                    ctx_tiles[(p, qb)] = psB.tile(
                        [128, 512], F32, tag="ctx", name=f"cx_{p[0]}_{p[1]}_{qb}"
                    )
                return ctx_tiles[(p, qb)]

            def make_steps(p, pas):
                h, b = p
                qlo, qhi = (0, 1024) if pas == 0 else (1024, 2048)
                kcs = range(8) if pas == 0 else range(KC)
                out = []
                for kc in kcs:
                    out.append((p, kc, qlo, qhi))
                return out

            # stream entries: ("kc", step) | ("a2a", b) | ("outproj", args)
            # Batch-aligned pair groups, zipped 1:1 so consecutive score
            # matmuls alternate h0/h64 row-groups (concurrent subarrays keep
            # the PE warm); ctx matmuls follow in same-shape blocks.
            # Batch 0's out-proj halves are inserted far enough into batch
            # 1's pass-1 stream that the AllToAll has completed -- the PE
            # queue is FIFO, so a premature out-proj matmul would block all
            # attention work queued behind it.
            stream = []
            for pas in (0, 1):
                for x, y in zip(make_steps((0, 0), pas), make_steps((1, 0), pas)):
                    stream.append(("kc", x))
                    stream.append(("kc", y))
            stream.append(("a2a", 0))
            for x, y in zip(make_steps((0, 1), 0), make_steps((1, 1), 0)):
                stream.append(("kc", x))
                stream.append(("kc", y))
            for x, y in zip(make_steps((0, 1), 1), make_steps((1, 1), 1)):
                stream.append(("kc", x))
                stream.append(("kc", y))
            stream.append(("a2a", 1))
            stream.append(("outproj", (0, 0)))
            stream.append(("outproj", (0, 1)))
            stream.append(("outproj", (1, 0, True)))
            stream.append(("writeback", (0, 256)))
            stream.append(("outproj", (1, 1, True)))

            pending = deque()

            def flush(n=None):
                if n is None:
                    while pending:
                        pending.popleft()()
                else:
                    for _ in range(n):
                        pending.popleft()()

            for kind, arg in stream:
                if kind == "kc":
                    p, kc, qlo, qhi = arg
                    h, b = p
                    at = scores_part(h, b, kc, qlo, qhi)
                    cps = {qb: get_ctx(p, qb) for qb in (qlo // 512, qlo // 512 + 1)}
                    pending.append(
                        lambda h=h, b=b, kc=kc, qlo=qlo, qhi=qhi, cps=cps, at=at:
                        ctx_part(h, b, lanes[(h, b)], kc, qlo, qhi, cps, at)
                    )
                    # flush ctx in same-shape blocks of two (one per head),
                    # lagging the scores by two rounds so exp() has landed
                    if len(pending) > 7:
                        flush(2)
                elif kind == "a2a":
                    flush()
                    ship_a2a(arg)
                elif kind == "outproj":
                    outproj_chunk(*arg)
                elif kind == "writeback":
                    writeback(*arg)

            flush()

            ctxA_free()
            outT_free()
            ctxT_free()
            kT_free()
            qT_free()

    nc.compile()
    return nc


def _get_program(with_bias: bool = False):
    key = ("nc", with_bias)
    if key not in _CACHED:
        _CACHED[key] = _build(with_bias)
    return _CACHED[key]


def kernel(x, mask, wq, bq, wk, bk, wv, bv, wo, bo):
    x = np.asarray(x, dtype=np.float32)
    mask = np.asarray(mask)
    bf = ml_dtypes.bfloat16

    with_bias = any(np.any(np.asarray(bb)) for bb in (bq, bk, bv, bo))
    nc = _get_program(with_bias)

    # [feature, batch*seq] activations
    xT = np.ascontiguousarray(x.reshape(T, D).T).astype(bf)
    woT = np.ascontiguousarray(np.asarray(wo).T).astype(bf)
    maskb = np.ascontiguousarray(
        np.where(np.asarray(mask).reshape(B * KC, 128), -10000.0, 0.0)
        .astype(np.float32)
        .T
    )
    in_maps = []
    for c in range(NCORES):
        fs = slice(c * F, (c + 1) * F)
        m = {
            "xT": xT,
            "wqT": np.ascontiguousarray(np.asarray(wq)[fs, :].T).astype(bf),
            "wkT": np.ascontiguousarray(np.asarray(wk)[fs, :].T).astype(bf),
            "wvT": np.ascontiguousarray(np.asarray(wv)[fs, :].T).astype(bf),
            "woT": woT,
            "maskb": maskb,
        }
        if with_bias:
            m["bq"] = np.asarray(bq)[fs].astype(bf).reshape(1, F)
            m["bk"] = np.asarray(bk)[fs].astype(bf).reshape(1, F)
            m["bv"] = np.asarray(bv)[fs].astype(bf).reshape(1, F)
            m["bo"] = np.asarray(bo).astype(bf).reshape(1, D)
        in_maps.append(m)

    res = bass_utils.run_bass_kernel_spmd(
        nc, in_maps, core_ids=list(range(NCORES)), trace=False
    )
    _CACHED["last_results"] = res

    # core c owns tokens [c*TOK, (c+1)*TOK) of each batch
    out = np.empty((B, S, D), dtype=np.float32)
    for c in range(NCORES):
        o = np.asarray(res.results[c]["outT"], dtype=np.float32)  # [D, 512]
        for b in range(B):
            out[b, c * TOK:(c + 1) * TOK, :] = o[:, b * TOK:(b + 1) * TOK].T
    return out


# revision 33
# speedup vs baseline: 1.0428x; 1.0428x over previous
"""Tensor-parallel multi-head attention for 8 Trainium2 NeuronCores.

Sharding (TP8 over heads): core c owns heads {2c, 2c+1} (128 q/k/v features)
and computes them for BOTH batch elements.  The out-projection is
TOKEN-sharded: after attention for batch b finishes, one 8-core AllToAll
(512 KB/core wire) redistributes context so core c holds ALL 1024 context
features for its 256-token slice of batch b; each core then computes the
full out-proj for its tokens with the complete (unsharded) Wo.  This
replaces the previous six serialized AllGathers (8 MB/core gathered,
~122 us of collective time) with two ~10 us AllToAlls, the first of which
fully overlaps batch 1's attention.

Per-core dataflow (activations kept transposed, [feature, token]):
  qT/kT/vT = W.T-chunks @ xT          (PE, bf16, fp32 PSUM accum)
  v        = PE-transpose(vT)          (with an appended ones-column)
  sT[k,q]  = kT-block.T @ qT           (causal: upper-right blocks skipped)
  aT       = exp(sT/8 + mask_bias)     (ACT from PSUM; safe without
                                        max-subtraction: scores ~ N(0,1))
  ctxT;sum = [v|1].T @ aT              (ones row gives the softmax denom)
  ctxT    *= 1/sum                     (per-q-block, as soon as its k-loop
                                        completes; batch-0 denominators
                                        broadcast on GpSimd -- its queue slot
                                        precedes every collective trigger --
                                        batch 1 via a bf16 PE matmul so it
                                        can never jam behind a collective's
                                        completion wait)
  AllToAll ctxT per batch; out-proj for my 256-token slice per batch

Engine queues are FIFO, so emission order is execution order per engine.
The attention stream is software-pipelined: the ctx matmuls of k-chunk step
N are emitted several steps behind its score matmuls, giving the Scalar
engine time to produce exp(scores) before the Tensor engine's queue reaches
the ctx matmul that consumes them (deep lag keeps the PE stream dense so
the HAM clock gate stays released).  Two (head, batch) pairs are
interleaved 2:1 so PSUM context accumulators fit alongside the
double-buffered score tiles.  Batch 0's out-proj is emitted after the
batch-1 AllToAll trigger so it fills the PE while that collective is in
flight; batch 1's out-proj runs as soon as its gather lands, streaming its
bf16 writebacks per out-chunk.  Collective staging DMAs are bulk
(not spread through the stream) and output writebacks are ordered behind
the gather loads, so nothing contends the SDMA engines while a collective
is on the wire; nothing except collective triggers rides the GpSimd queue,
whose completion-waits would otherwise jam work queued behind them.
Host side only reshapes/concatenates shards (dtype prep of inputs aside);
the gathered feature order is already the global head order, so no host
permutation is needed.
"""

import sys
from collections import deque

for _p in ("/opt/trn_rl_repo",):
    if _p not in sys.path:
        sys.path.append(_p)

import numpy as np
import ml_dtypes

import concourse.bass as bass  # noqa: F401
import concourse.mybir as mybir
import concourse.tile as tile
from concourse import bacc, bass_utils
from concourse.masks import make_identity, make_upper_triangular

BF16 = mybir.dt.bfloat16
F32 = mybir.dt.float32
Exp = mybir.ActivationFunctionType.Exp

B, S, D = 2, 2048, 1024
T = B * S            # 4096 tokens across batches
H, DH = 16, 64
NCORES = 8
HPC = H // NCORES    # heads per core = 2
F = HPC * DH         # features per core = 128
KC = S // 128        # 16 k-chunks per batch
TOK = S // NCORES    # tokens per core per batch for the out-proj = 256

PAIRS = [(0, 0), (1, 0), (0, 1), (1, 1)]

_CACHED = {}


def _build(with_bias: bool):
    nc = bacc.Bacc(
        "TRN2",
        target_bir_lowering=False,
        debug=False,
        enable_asserts=True,
        num_devices=NCORES,
    )
    xT_d = nc.dram_tensor("xT", [D, T], BF16, kind="ExternalInput").ap()
    wqT_d = nc.dram_tensor("wqT", [D, F], BF16, kind="ExternalInput").ap()
    wkT_d = nc.dram_tensor("wkT", [D, F], BF16, kind="ExternalInput").ap()
    wvT_d = nc.dram_tensor("wvT", [D, F], BF16, kind="ExternalInput").ap()
    woT_d = nc.dram_tensor("woT", [D, D], BF16, kind="ExternalInput").ap()
    b_d = {}
    if with_bias:
        for nm, width in (("bq", F), ("bk", F), ("bv", F), ("bo", D)):
            b_d[nm] = nc.dram_tensor(nm, [1, width], BF16, kind="ExternalInput").ap()
    maskb_d = nc.dram_tensor("maskb", [128, B * KC], F32, kind="ExternalInput").ap()
    outT_d = nc.dram_tensor("outT", [D, B * TOK], BF16, kind="ExternalOutput").ap()

    with tile.TileContext(nc) as tc:
        with (
            tc.tile_pool(name="singles", bufs=1) as sg,
            tc.tile_pool(name="att", bufs=8) as att_pool,
            tc.tile_pool(name="psA", bufs=2, space="PSUM") as psA,
            tc.tile_pool(name="psB", bufs=4, space="PSUM") as psB,
            tc.tile_pool(name="dram", bufs=1, space="DRAM") as dram,
        ):
            # ---- constants -------------------------------------------------
            ident = sg.tile([128, 128], BF16, name="ident")
            make_identity(nc, ident)
            trimask = sg.tile([128, 128], BF16, name="trimask")
            make_upper_triangular(nc, trimask, val=1.0, diag=True)
            ones64b = sg.tile([1, 64], BF16, name="ones64b")
            nc.vector.memset(ones64b, 1.0)
            if with_bias:
                ones512 = sg.tile([1, 512], BF16, name="ones512")
                nc.vector.memset(ones512, 1.0)

            # ---- load inputs (split for early start) -----------------------
            w_sb = {}
            for nm, dd, fo in (("v", wvT_d, F), ("k", wkT_d, F), ("q", wqT_d, F)):
                w_sb[nm] = sg.tile([128, 8, fo], BF16, name=f"w{nm}T_sb")
                nc.sync.dma_start(w_sb[nm], dd.rearrange("(o p) f -> p o f", p=128))
            b_sb = {}
            if with_bias:
                for nm, width in (("bq", F), ("bk", F), ("bv", F), ("bo", D)):
                    b_sb[nm] = sg.tile([1, width], BF16, name=f"{nm}_sb")
                    nc.sync.dma_start(b_sb[nm], b_d[nm])

            # persistent activations first, xT last (freed first: LIFO stack)
            qT_sb, qT_free = tc.tile([128, T], BF16, name="qT_sb")
            kT_sb, kT_free = tc.tile([128, T], BF16, name="kT_sb")
            ctxT_sb, ctxT_free = tc.tile([64, HPC, T], BF16, name="ctxT_sb")
            vT_sb, vT_free = tc.tile([128, T], BF16, name="vT_sb")
            xT_sb, xT_free = tc.tile([128, 8, T], BF16, name="xT_sb")
            xT_r = xT_d.rearrange("(o p) f -> p o f", p=128)
            # ki-major for the first half-pair (projection consumes ki-order),
            # second half-pair afterwards (its projections run later)
            for hp in range(2):
                for ki in range(8):
                    for half in (2 * hp, 2 * hp + 1):
                        cs = half * 1024
                        nc.sync.dma_start(
                            xT_sb[:, ki, cs:cs + 1024], xT_r[:, ki, cs:cs + 1024]
                        )
            # maskb (attention-phase) and Wo (mid-kernel) land after x
            maskb_sb = sg.tile([128, B * KC], F32, name="maskb_sb")
            nc.sync.dma_start(maskb_sb, maskb_d)
            w_sb["o"] = sg.tile([128, 8, D], BF16, name="woT_sb")
            nc.sync.dma_start(w_sb["o"], woT_d.rearrange("(o p) f -> p o f", p=128))

            # ---- HAM warmup: dependency-free matmuls fill the otherwise
            # idle PE while the first x/weight DMAs stream in, so the clock
            # gate is already released when the projections start ----------
            warm = psA.tile([128, 512], F32, tag="work", name="warm")
            for _ in range(64):
                nc.tensor.matmul(
                    warm[:, 0:128], lhsT=ident, rhs=ident, start=True, stop=True
                )

            # ---- projections (ki-outer: 4 matmuls per weight load) ---------
            def project(w, bias, dst, which, hp):
                pss = [
                    psA.tile(
                        [128, 1024], F32, tag="work",
                        name=f"p_{which}_{2 * hp + i}",
                    )
                    for i in range(2)
                ]
                for ki in range(8):
                    for i in range(2):
                        half = 2 * hp + i
                        for nb in range(2):
                            cs = half * 1024 + nb * 512
                            nc.tensor.matmul(
                                pss[i][:, nb * 512:nb * 512 + 512],
                                lhsT=w[:, ki, :],
                                rhs=xT_sb[:, ki, cs:cs + 512],
                                start=(ki == 0),
                                stop=(ki == 7 and not with_bias),
                            )
                for i in range(2):
                    half = 2 * hp + i
                    if with_bias:
                        for nb in range(2):
                            nc.tensor.matmul(
                                pss[i][:, nb * 512:nb * 512 + 512],
                                lhsT=bias[0:1, :],
                                rhs=ones512[0:1, :],
                                start=False,
                                stop=True,
                            )
                    nc.vector.tensor_copy(
                        dst[:, half * 1024:half * 1024 + 1024], pss[i]
                    )

            def project_vk(hp):
                vt = [
                    psA.tile(
                        [128, 1024], F32, tag="work", name=f"p_v_{2 * hp + i}"
                    )
                    for i in range(2)
                ]
                kt = [
                    psB.tile(
                        [128, 512], F32, tag="ctx", name=f"p_k_{2 * hp}_{j}"
                    )
                    for j in range(4)
                ]
                for ki in range(8):
                    for i in range(2):
                        half = 2 * hp + i
                        for nb in range(2):
                            cs = half * 1024 + nb * 512
                            nc.tensor.matmul(
                                vt[i][:, nb * 512:nb * 512 + 512],
                                lhsT=w_sb["v"][:, ki, :],
                                rhs=xT_sb[:, ki, cs:cs + 512],
                                start=(ki == 0),
                                stop=(ki == 7 and not with_bias),
                            )
                            nc.tensor.matmul(
                                kt[2 * i + nb],
                                lhsT=w_sb["k"][:, ki, :],
                                rhs=xT_sb[:, ki, cs:cs + 512],
                                start=(ki == 0),
                                stop=(ki == 7 and not with_bias),
                            )
                for i in range(2):
                    half = 2 * hp + i
                    for nb in range(2):
                        cs = half * 1024 + nb * 512
                        if with_bias:
                            nc.tensor.matmul(
                                vt[i][:, nb * 512:nb * 512 + 512],
                                lhsT=b_sb["bv"][0:1, :],
                                rhs=ones512[0:1, :],
                                start=False,
                                stop=True,
                            )
                            nc.tensor.matmul(
                                kt[2 * i + nb],
                                lhsT=b_sb["bk"][0:1, :],
                                rhs=ones512[0:1, :],
                                start=False,
                                stop=True,
                            )
                        nc.vector.tensor_copy(
                            kT_sb[:, cs:cs + 512], kt[2 * i + nb]
                        )
                    nc.vector.tensor_copy(
                        vT_sb[:, half * 1024:half * 1024 + 1024], vt[i]
                    )

            # v+k interleaved (DMA-paced), then q (x already resident)
            for hp in range(2):
                project_vk(hp)
                project(w_sb["q"], b_sb.get("bq"), qT_sb, "q", hp)

            # ---- transpose v into [token, feat] blocks with ones column ----
            v_ones = sg.tile([128, B * KC, HPC, DH + 1], BF16, name="v_ones")
            nc.vector.memset(v_ones, 1.0)
            for tb in range(B * KC):
                pt = psB.tile([128, 128], BF16, tag="ctx", name=f"vt_{tb}")
                nc.tensor.transpose(pt, vT_sb[:, tb * 128:tb * 128 + 128], ident)
                for h in range(HPC):
                    nc.vector.tensor_copy(
                        v_ones[:, tb, h, 0:DH], pt[:, h * 64:h * 64 + 64]
                    )

            xT_free()
            vT_free()

            # ---- attention -------------------------------------------------
            sums_b = sg.tile([1, 2 * S], BF16, name="sums_b")
            sums_f = sg.tile([1, 2 * S], F32, name="sums_f")
            rec1_sb = sg.tile([1, 2 * S], F32, name="rec1_sb")
            rec_sb = sg.tile([64, 2048], F32, name="rec_sb")
            # out-proj token columns: [0:256) = my batch-0 token slice,
            # [256:512) = my batch-1 token slice
            outT_sb, outT_free = tc.tile([128, 8, B * TOK], BF16, name="outT_sb")
            ctxA_sb, ctxA_free = tc.tile([128, 8, B * TOK], BF16, name="ctxA_sb")

            # ships: key = batch index, one AllToAll per batch (256
            # tok/core); the batch-0 ship fully overlaps batch-1 attention
            SHIPS = {0: (0, 0, S), 1: (1, 0, S)}
            a2a_in = {}
            a2a_out = {}
            for key, (_b, _q0, qn) in SHIPS.items():
                tk = qn // NCORES
                a2a_in[key] = dram.tile([NCORES * F, tk], BF16, name=f"a2ai_{key}")
                a2a_out[key] = dram.tile([NCORES * F, tk], BF16, name=f"a2ao_{key}")

            # normalize is split in two: the sums copy is emitted with the
            # final ctx matmul; the broadcast matmul + reciprocal + scale run
            # one pipeline round later so the PE never waits on the Vector
            # queue.  The scaled ctxT q-block is shipped to the AllToAll
            # staging buffer immediately, so the collective trigger only
            # waits on the final 64 KB piece.
            def normalize_a(h, b, lane, qb, ctx_tile):
                so = lane * S + qb * 512
                if b == 0:
                    nc.vector.tensor_copy(
                        sums_f[0:1, so:so + 512], ctx_tile[DH:DH + 1, :]
                    )
                    nc.vector.reciprocal_approx_fast(
                        rec1_sb[0:1, so:so + 512], sums_f[0:1, so:so + 512]
                    )
                else:
                    nc.vector.tensor_copy(
                        sums_b[0:1, so:so + 512], ctx_tile[DH:DH + 1, :]
                    )

            def normalize_b(h, b, lane, qb, ctx_tile):
                t0 = b * S
                so = lane * S + qb * 512
                ro = lane * 1024 + (qb % 2) * 512
                # batch 0: GpSimd partition_broadcast (its queue slot is
                # ahead of every collective trigger, so it can never jam).
                # batch 1: broadcast via a tiny bf16 matmul -- its GpSimd
                # slot would sit behind the batch-0 AllToAll trigger's
                # completion wait, jamming the normalize chain whenever that
                # collective runs long (measured 17 us PE stalls).
                if b == 0:
                    nc.gpsimd.partition_broadcast(
                        rec_sb[:, ro:ro + 512], rec1_sb[0:1, so:so + 512]
                    )
                else:
                    bc = psA.tile(
                        [128, 512], F32, tag="work", name=f"bc_{h}_{b}_{qb}"
                    )
                    nc.tensor.matmul(
                        bc[0:64, :],
                        lhsT=ones64b[0:1, :],
                        rhs=sums_b[0:1, so:so + 512],
                        start=True,
                        stop=True,
                    )
                    nc.vector.reciprocal_approx_fast(
                        rec_sb[:, ro:ro + 512], bc[0:64, :]
                    )
                nc.vector.tensor_mul(
                    ctxT_sb[:, h, t0 + qb * 512:t0 + qb * 512 + 512],
                    ctx_tile[0:DH, :],
                    rec_sb[:, ro:ro + 512],
                )

            def scores_part(h, b, kc, qlo, qhi):
                """Emit score matmuls + exp for one k-chunk; returns the
                attention-weights tile for the ctx part."""
                po = 64 * h
                t0 = b * S
                q0 = kc * 128
                lo = max(q0, qlo)
                w = qhi - lo
                kT_blk = kT_sb[po:po + 64, t0 + q0:t0 + q0 + 128]
                st = psA.tile(
                    [128, 1024], F32, tag="work", name=f"st_{h}_{b}_{kc}_{qlo}"
                )
                c = lo
                while c < qhi:
                    c2 = min(qhi, (c // 512 + 1) * 512)
                    nc.tensor.matmul(
                        st[:, c - qlo:c2 - qlo],
                        lhsT=kT_blk,
                        rhs=qT_sb[po:po + 64, t0 + c:t0 + c2],
                        start=True,
                        stop=True,
                    )
                    c = c2
                at = att_pool.tile([128, 1024], BF16, tag="att")
                nc.scalar.activation(
                    at[:, 0:w],
                    st[:, lo - qlo:qhi - qlo],
                    Exp,
                    bias=maskb_sb[:, b * KC + kc:b * KC + kc + 1],
                    scale=0.125,
                )
                if lo == q0:  # diagonal 128-block: causal interior
                    nc.vector.tensor_mul(at[:, 0:128], at[:, 0:128], trimask)
                return at

            def ctx_part(h, b, lane, kc, qlo, qhi, ctx_ps, at):
                t0 = b * S
                q0 = kc * 128
                lo = max(q0, qlo)
                c = lo
                while c < qhi:
                    qb = c // 512
                    c2 = min(qhi, (qb + 1) * 512)
                    nc.tensor.matmul(
                        ctx_ps[qb][0:DH + 1, c - qb * 512:c2 - qb * 512],
                        lhsT=v_ones[:, b * KC + kc, h, :],
                        rhs=at[:, c - lo:c2 - lo],
                        start=(kc == 0),
                        stop=(kc == 4 * qb + 3),
                    )
                    c = c2
                if kc >= 3 and (kc - 3) % 4 == 0:
                    qb_done = (kc - 3) // 4
                    if qlo <= qb_done * 512 < qhi:
                        normalize_a(h, b, lane, qb_done, ctx_ps[qb_done])
                        pending.append(
                            lambda h=h, b=b, lane=lane, qb=qb_done,
                            ct=ctx_ps[qb_done]: normalize_b(h, b, lane, qb, ct)
                        )

            # out-proj column ranges per ship key
            OCOL = {0: 0, 1: 256}

            def ship_a2a(key):
                # scatter my ctxT for this token range into 8 shards (one per
                # peer): shard j = my 128 features for its j-th token slice.
                # Bulk DMA here (not staged per q-block): staging DMAs spread
                # through the attention stream land inside an earlier ship's
                # AllToAll window and contend for the SDMA engines, measured
                # to cut the collective's wire rate ~4x.
                b, q0, qn = SHIPS[key]
                tk = qn // NCORES
                dst = a2a_in[key].rearrange(
                    "(j h p) t -> p h j t", j=NCORES, h=HPC, p=DH
                )
                src = ctxT_sb[:, :, b * S + q0:b * S + q0 + qn].rearrange(
                    "p h (j t) -> p h j t", j=NCORES
                )
                nc.sync.dma_start(dst[:, 0], src[:, 0])
                nc.scalar.dma_start(dst[:, 1], src[:, 1])
                nc.gpsimd.collective_compute(
                    "AllToAll",
                    mybir.AluOpType.bypass,
                    replica_groups=[list(range(NCORES))],
                    ins=[a2a_in[key].opt()],
                    outs=[a2a_out[key].opt()],
                )
                # my slice: all 1024 features (global head order) x tk
                # tokens; the final ship's load is split so the out-proj can
                # start accumulating after the first half lands
                r = a2a_out[key].rearrange("(k p) t -> p k t", p=128)
                co = OCOL[key]
                nc.sync.dma_start(ctxA_sb[:, :, co:co + tk], r)

            outv = outT_d.rearrange("(o p) t -> p o t", p=128)

            def outproj_chunk(key, half, stream_wb=False):
                _b, _q0, qn = SHIPS[key]
                tk = qn // NCORES
                co = OCOL[key]
                ps = psA.tile(
                    [128, 4, tk], F32, tag="work", name=f"op_{key}_{half}"
                )
                tslc = slice(co, co + tk)
                for i in range(4):
                    oc = half * 4 + i
                    for ki in range(8):
                        nc.tensor.matmul(
                            ps[:, i, :],
                            lhsT=w_sb["o"][:, ki, oc * 128:oc * 128 + 128],
                            rhs=ctxA_sb[:, ki, tslc],
                            start=(ki == 0),
                            stop=(ki == 7 and not with_bias),
                        )
                    if with_bias:
                        nc.tensor.matmul(
                            ps[:, i, :],
                            lhsT=b_sb["bo"][0:1, oc * 128:oc * 128 + 128],
                            rhs=ones512[0:1, 0:tk],
                            start=False,
                            stop=True,
                        )
                    nc.vector.tensor_copy(outT_sb[:, oc, tslc], ps[:, i, :])
                    if stream_wb:
                        nc.sync.dma_start(
                            outv[:, oc, tslc], outT_sb[:, oc, tslc]
                        )

            def writeback(lo, hi):
                # sync queue: ordered behind the ctxA gather loads, so the
                # writes never overlap an in-flight collective
                nc.sync.dma_start(outv[:, :, lo:hi], outT_sb[:, :, lo:hi])

            # ---- build the interleaved, software-pipelined stream ---------
            lanes = {p: i % 2 for i, p in enumerate(PAIRS)}
            ctx_tiles = {}

            def get_ctx(p, qb):
                if (p, qb) not in ctx_tiles:
                    ctx_tiles[(p, qb)] = psB.tile(
                        [128, 512], F32, tag="ctx", name=f"cx_{p[0]}_{p[1]}_{qb}"
                    )
                return ctx_tiles[(p, qb)]

            def make_steps(p, pas):
                h, b = p
                qlo, qhi = (0, 1024) if pas == 0 else (1024, 2048)
                kcs = range(8) if pas == 0 else range(KC)
                out = []
                for kc in kcs:
                    out.append((p, kc, qlo, qhi))
                return out

            # stream entries: ("kc", step) | ("a2a", b) | ("outproj", args)
            # Batch-aligned pair groups, zipped 1:1 so consecutive score
            # matmuls alternate h0/h64 row-groups (concurrent subarrays keep
            # the PE warm); ctx matmuls follow in same-shape blocks.
            # Batch 0's out-proj halves are inserted far enough into batch
            # 1's pass-1 stream that the AllToAll has completed -- the PE
            # queue is FIFO, so a premature out-proj matmul would block all
            # attention work queued behind it.
            stream = []
            for pas in (0, 1):
                for x, y in zip(make_steps((0, 0), pas), make_steps((1, 0), pas)):
                    stream.append(("kc", x))
                    stream.append(("kc", y))
            stream.append(("a2a", 0))
            for x, y in zip(make_steps((0, 1), 0), make_steps((1, 1), 0)):
                stream.append(("kc", x))
                stream.append(("kc", y))
            for x, y in zip(make_steps((0, 1), 1), make_steps((1, 1), 1)):
                stream.append(("kc", x))
                stream.append(("kc", y))
            stream.append(("a2a", 1))
            stream.append(("outproj", (0, 0)))
            stream.append(("outproj", (0, 1)))
            stream.append(("outproj", (1, 0, True)))
            stream.append(("writeback", (0, 256)))
            stream.append(("outproj", (1, 1, True)))

            pending = deque()

            def flush(n=None):
                if n is None:
                    while pending:
                        pending.popleft()()
                else:
                    for _ in range(n):
                        pending.popleft()()

            for kind, arg in stream:
                if kind == "kc":
                    p, kc, qlo, qhi = arg
                    h, b = p
                    at = scores_part(h, b, kc, qlo, qhi)
                    cps = {qb: get_ctx(p, qb) for qb in (qlo // 512, qlo // 512 + 1)}
                    pending.append(
                        lambda h=h, b=b, kc=kc, qlo=qlo, qhi=qhi, cps=cps, at=at:
                        ctx_part(h, b, lanes[(h, b)], kc, qlo, qhi, cps, at)
                    )
                    # flush ctx in same-shape blocks of two (one per head),
                    # lagging the scores by two rounds so exp() has landed
                    if len(pending) > 7:
                        flush(2)
                elif kind == "a2a":
                    flush()
                    ship_a2a(arg)
                elif kind == "outproj":
                    outproj_chunk(*arg)
                elif kind == "writeback":
                    writeback(*arg)

            flush()

            ctxA_free()
            outT_free()
            ctxT_free()
            kT_free()
            qT_free()

    nc.compile()
    return nc


def _get_program(with_bias: bool = False):
    key = ("nc", with_bias)
    if key not in _CACHED:
        _CACHED[key] = _build(with_bias)
    return _CACHED[key]


def kernel(x, mask, wq, bq, wk, bk, wv, bv, wo, bo):
    x = np.asarray(x, dtype=np.float32)
    mask = np.asarray(mask)
    bf = ml_dtypes.bfloat16

    with_bias = any(np.any(np.asarray(bb)) for bb in (bq, bk, bv, bo))
    nc = _get_program(with_bias)

    # [feature, batch*seq] activations
    xT = np.ascontiguousarray(x.reshape(T, D).T).astype(bf)
    woT = np.ascontiguousarray(np.asarray(wo).T).astype(bf)
    maskb = np.ascontiguousarray(
        np.where(np.asarray(mask).reshape(B * KC, 128), -10000.0, 0.0)
        .astype(np.float32)
        .T
    )
    in_maps = []
    for c in range(NCORES):
        fs = slice(c * F, (c + 1) * F)
        m = {
            "xT": xT,
            "wqT": np.ascontiguousarray(np.asarray(wq)[fs, :].T).astype(bf),
            "wkT": np.ascontiguousarray(np.asarray(wk)[fs, :].T).astype(bf),
            "wvT": np.ascontiguousarray(np.asarray(wv)[fs, :].T).astype(bf),
            "woT": woT,
            "maskb": maskb,
        }
        if with_bias:
            m["bq"] = np.asarray(bq)[fs].astype(bf).reshape(1, F)
            m["bk"] = np.asarray(bk)[fs].astype(bf).reshape(1, F)
            m["bv"] = np.asarray(bv)[fs].astype(bf).reshape(1, F)
            m["bo"] = np.asarray(bo).astype(bf).reshape(1, D)
        in_maps.append(m)

    res = bass_utils.run_bass_kernel_spmd(
        nc, in_maps, core_ids=list(range(NCORES)), trace=False
    )
    _CACHED["last_results"] = res

    # core c owns tokens [c*TOK, (c+1)*TOK) of each batch
    out = np.empty((B, S, D), dtype=np.float32)
    for c in range(NCORES):
        o = np.asarray(res.results[c]["outT"], dtype=np.float32)  # [D, 512]
        for b in range(B):
            out[b, c * TOK:(c + 1) * TOK, :] = o[:, b * TOK:(b + 1) * TOK].T
    return out
